# revision 38
# baseline (speedup 1.0000x reference)
"""Trainium2 Bass kernel for nn_EpisodicMemoryModule.

Strategy
--------
Math restructure: inside each episode's scan, the gate chain
z -> g1 -> g depends only on (c_t, m, q) -- never on h. So G[t] for all
timesteps is precomputed with large batched matmuls, as is the
x-dependent half of the attention GRU (gi_att = c @ att_Wih.T, which is
even episode-invariant). The only sequential work left per scan step is
gh = h @ att_Whh.T plus elementwise gates.

Sharding: data-parallel over batch B=64 across 8 cores (8 rows/core),
no inter-core communication. All weights are pre-transposed on the host
into contraction-major layout (features on partitions) and pre-cast:
bf16 for everything except att_Whh.T which is fp8e4m3 scaled by 32
(the GRU recurrence is contractive; end-to-end rel err ~1e-3).

Within a core everything is feature-major ([d partitions, (t,b) free]),
which keeps the per-step gate elementwise work on fully-occupied
128-partition tiles.
"""

import itertools
import sys

sys.path.insert(0, "/opt/trn_rl_repo")

_DONE = object()

import numpy as np
import ml_dtypes

import concourse.bass as bass
import concourse.mybir as mybir
from concourse.bass_utils import run_bass_kernel_spmd
from concourse.tile import TileContext
import bass_rust
from bass_rust import ScopedClock

T, B, D = 128, 64, 1024
NCORES = 8
BL = B // NCORES          # 8 batch rows per core
ROWS = T * BL             # 1024 rows per core
RB = 512                  # row-block for precompute matmuls
NRB = ROWS // RB          # 2
KC = D // 128             # 8 contraction chunks
S_WHH = 32.0              # fp8 scale for att_Whh

F32 = mybir.dt.float32
BF16 = mybir.dt.bfloat16
FP8 = mybir.dt.float8e4
AF = mybir.ActivationFunctionType
ALU = mybir.AluOpType

bf16_np = ml_dtypes.bfloat16
fp8_np = ml_dtypes.float8_e4m3fn


class _TC(TileContext):
    """TileContext whose final drain splits multi-sem waits (the walrus in
    this environment accepts only one sync wait per instruction)."""

    def _drain_and_barrier(self, tick_clock, wait_clock):
        drain_inst = self.nc.sync.drain()
        wait_clock.add_sem_waits(
            drain_inst.ins, ScopedClock({None: tick_clock.global_clock})
        )
        si = drain_inst.ins.sync_info
        if si is not None and si.on_wait and len(si.on_wait) > 1:
            waits = list(si.on_wait)
            drain_inst.ins.sync_info = bass_rust.SyncInfo(
                on_wait=[waits[0]], on_update=list(si.on_update or [])
            )
            for w in waits[1:]:
                d = self.nc.sync.drain()
                d.ins.sync_info = bass_rust.SyncInfo(on_wait=[w], on_update=[])
        self.nc.all_engine_barrier()
        assert self.sems is not None
        popped = self.nc._tile_sem_poison_stack.pop()
        assert popped is self._sem_poison
        self.nc.clear_and_free_semaphores(list(self.sems.allocated().values()))
        self.nc.all_engine_barrier()


def _split_multiwait(nc):
    """Split >1-wait instructions into single-wait NoOps + instruction."""
    nfix = 0
    for f in nc.m.functions:
        for bb in f.blocks:
            insts = list(bb.instructions)
            out = []
            changed = False
            for inst in insts:
                si = inst.sync_info
                if si and si.on_wait and len(si.on_wait) > 1:
                    waits = list(si.on_wait)
                    for i, w in enumerate(waits[:-1]):
                        nop = mybir.InstNoOp(
                            name=f"I-waitfix-{nfix}-{i}", ins=[], outs=[]
                        )
                        nop.engine = inst.engine
                        nop.sync_info = bass_rust.SyncInfo(on_wait=[w], on_update=[])
                        out.append(nop)
                    inst.sync_info = bass_rust.SyncInfo(
                        on_wait=[waits[-1]], on_update=list(si.on_update or [])
                    )
                    nfix += 1
                    changed = True
                out.append(inst)
            if changed:
                bb.instructions = out
    return nfix


def _build(t_steps=T, split_waits=True, phases=('gi', 'P', 'zg', 'scan', 'mem', 'small')):
    """Build the per-core Bass module (SPMD; every core runs the same
    program on its own batch shard)."""
    nc = bass.Bass()
    P = nc.declare_dram_parameter

    # Per-core activations (feature-major) and vectors.
    cT = P("cT", [D, ROWS], F32, isOutput=False)            # c.T shard
    qT = P("qT", [128, KC * BL], F32, isOutput=False)       # q.T folded (p,(k,b))
    # Weights, contraction-major (in-features on partitions).
    w1T = P("w1T", [9 * D, D], BF16, isOutput=False)        # W1.T
    w12T = P("w12T", [D, D], BF16, isOutput=False)          # (W1_1+W1_2).T
    wbT = P("wbT", [D, D], BF16, isOutput=False)            # Wb.T
    w2T = P("w2T", [D, D], BF16, isOutput=False)            # W2.T
    aWihT = P("aWihT", [D, 3 * D], BF16, isOutput=False)    # att_Wih.T
    aWhhT8 = P("aWhhT8", [D, 3 * D], FP8, isOutput=False)   # att_Whh.T * 32 fp8
    mWihT = P("mWihT", [D, 3 * D], BF16, isOutput=False)
    mWhhT = P("mWhhT", [D, 3 * D], BF16, isOutput=False)
    bZ = P("bZ", [128, KC], F32, isOutput=False)            # W1_b per (p, m)
    bG = P("bG", [128, KC], F32, isOutput=False)            # W2_b
    bA = P("bA", [128, 3 * KC], F32, isOutput=False)        # att bih(+bhh for r,z)
    bM = P("bM", [128, 3 * KC], F32, isOutput=False)        # mem bih(+bhh for r,z)
    out = P("out", [128, KC * BL], F32, isOutput=True)      # m2.T folded

    with _TC(nc) as tc:
        pool = tc.alloc_tile_pool(name="res", bufs=1)
        stream = tc.alloc_tile_pool(name="stream", bufs=6)
        scratch = tc.alloc_tile_pool(name="scratch", bufs=3)

        # ---- resident loads -------------------------------------------------
        cT_sb = pool.tile([128, KC * ROWS], F32, tag="cT")     # 32 KB/par
        for k in range(KC):
            nc.sync.dma_start(
                out=cT_sb[:, k * ROWS:(k + 1) * ROWS],
                in_=cT[k * 128:(k + 1) * 128, :],
            )
        qT_sb = pool.tile([128, KC * BL], F32, tag="qT")
        nc.sync.dma_start(out=qT_sb[:, :], in_=qT[:, :])
        whh_sb = pool.tile([128, KC * 3 * D], FP8, tag="whh")  # 24 KB/par
        for k in range(KC):
            nc.sync.dma_start(
                out=whh_sb[:, k * 3 * D:(k + 1) * 3 * D],
                in_=aWhhT8[k * 128:(k + 1) * 128, :],
            )
        bZ_sb = pool.tile([128, KC], F32, tag="bZ")
        nc.sync.dma_start(out=bZ_sb[:, :], in_=bZ[:, :])
        bG_sb = pool.tile([128, KC], F32, tag="bG")
        nc.sync.dma_start(out=bG_sb[:, :], in_=bG[:, :])
        bA_sb = pool.tile([128, 3 * KC], F32, tag="bA")
        nc.sync.dma_start(out=bA_sb[:, :], in_=bA[:, :])
        bM_sb = pool.tile([128, 3 * KC], F32, tag="bM")
        nc.sync.dma_start(out=bM_sb[:, :], in_=bM[:, :])

        qb_sb = pool.tile([128, KC * BL], BF16, tag="qb")
        nc.vector.tensor_copy(qb_sb[:, :], qT_sb[:, :])

        gi_sb = pool.tile([128, 3 * KC * ROWS], BF16, tag="gi")  # 48 KB/par
        p_sb = pool.tile([128, KC * ROWS], BF16, tag="P")        # 16 KB/par
        g_sb = pool.tile([128, KC * ROWS], BF16, tag="G")        # 16 KB/par

        def small_matmul(wT_dram, vec_sb, out_sb, bias=None, accum_from=None,
                         tagp="smallp"):
            """out.T[dout, BL] = W @ vec  (all feature-major [128, KC*BL]).
            One psum bank, k-outer single accumulation group, chunked weight
            DMA, fused bias/accum adds."""
            ps = tc.alloc_tile_pool(name="smallps", bufs=1, space="PSUM")
            pt = ps.tile([128, KC * BL], F32, tag=tagp, name=f"spt_{tagp}")
            for k in range(KC):
                wt = stream.tile([128, D], BF16, tag="w1w", name=f"sw_{tagp}{k}")
                nc.sync.dma_start(
                    out=wt[:, :], in_=wT_dram[k * 128:(k + 1) * 128, :])
                for m in range(KC):
                    nc.tensor.matmul(
                        pt[:, m * BL:(m + 1) * BL],
                        wt[:, m * 128:(m + 1) * 128],
                        vec_sb[:, k * BL:(k + 1) * BL],
                        start=(k == 0 and m == 0),
                        stop=(k == KC - 1 and m == KC - 1),
                    )
            src0 = pt[:, :]
            if bias is not None:
                nc.vector.tensor_tensor(
                    out_sb[:, :].rearrange("p (m b) -> p m b", b=BL),
                    src0.rearrange("p (m b) -> p m b", b=BL),
                    bias[:, :].unsqueeze(2).broadcast_to([128, KC, BL]),
                    ALU.add)
                src0 = out_sb[:, :]
            if accum_from is not None:
                nc.vector.tensor_tensor(
                    out_sb[:, :], src0, accum_from[:, :], ALU.add)
            elif bias is None:
                nc.vector.tensor_copy(out_sb[:, :], src0)
            ps.release()

        # ---- Wbq = q @ Wb.T ; qc1 = q @ (W1_1+W1_2).T + W1_b ---------------
        wbq_sb = pool.tile([128, KC * BL], F32, tag="wbq")
        qc1_sb = pool.tile([128, KC * BL], F32, tag="qc1")
        if 'small' in phases:
            small_matmul(wbT, qb_sb, wbq_sb)
            small_matmul(w12T, qb_sb, qc1_sb, bias=bZ_sb)
        else:
            nc.vector.memset(wbq_sb[:, :], 0.0)
            nc.vector.memset(qc1_sb[:, :], 0.0)

        def cast_c(k, ro, rl):
            c8 = scratch.tile([128, rl], BF16, tag="c8", bufs=2,
                              name=f"c8_{k}_{ro}")
            nc.vector.tensor_copy(
                c8[:, :], cT_sb[:, k * ROWS + ro: k * ROWS + ro + rl])
            return c8

        def make_blocks(k, ro, rl, vecf_sb, wbv_sb, tagsfx):
            """m-dependent blocks (c*v, |c-v|, c*Wbv) for chunk k, rows
            [ro, ro+rl); bf16 [128, rl] each."""
            cslice = cT_sb[:, k * ROWS + ro: k * ROWS + ro + rl]
            cview = cslice.rearrange("p (t b) -> p t b", b=BL)
            vv = vecf_sb[:, k * BL:(k + 1) * BL].unsqueeze(1).broadcast_to(
                [128, rl // BL, BL])
            wv = wbv_sb[:, k * BL:(k + 1) * BL].unsqueeze(1).broadcast_to(
                [128, rl // BL, BL])
            cm = scratch.tile([128, rl], BF16, tag="blk_cm", bufs=2,
                              name=f"cm{tagsfx}")
            nc.vector.tensor_tensor(
                cm[:, :].rearrange("p (t b) -> p t b", b=BL), cview, vv, ALU.mult)
            tmp = scratch.tile([128, rl], F32, tag="blk_tmp", bufs=2,
                               name=f"bt{tagsfx}")
            nc.vector.tensor_tensor(
                tmp[:, :].rearrange("p (t b) -> p t b", b=BL), cview, vv,
                ALU.subtract)
            am = scratch.tile([128, rl], BF16, tag="blk_am", bufs=2,
                              name=f"am{tagsfx}")
            nc.scalar.activation(am[:, :], tmp[:, :], AF.Abs)
            wm = scratch.tile([128, rl], BF16, tag="blk_wm", bufs=2,
                              name=f"wm{tagsfx}")
            nc.vector.tensor_tensor(
                wm[:, :].rearrange("p (t b) -> p t b", b=BL), cview, wv, ALU.mult)
            return cm, am, wm

        def pass_P(vecf_sb, wbv_sb, spans):
            """Shared Z part: P = c@W1_0.T + (c*q)@W1_3.T + |c-q|@W1_5.T
            + (c*Wbq)@W1_7.T  -> p_sb (bf16). Generator: yields between
            sub-chunks so callers can interleave emission with the scan."""
            for si, (ro, rl) in enumerate(spans):
                ps = tc.alloc_tile_pool(name=f"Pps{ro}", bufs=1, space="PSUM")
                zp2 = [ps.tile([128, 2 * rl], F32, tag=f"zp{m2}",
                               name=f"pp{ro}_{m2}")
                       for m2 in range(KC // 2)]
                zps = [zp2[m // 2][:, (m % 2) * rl:(m % 2 + 1) * rl]
                       for m in range(KC)]
                for k in range(KC):
                    c8 = cast_c(k, ro, rl)
                    cm, am, wm = make_blocks(k, ro, rl, vecf_sb, wbv_sb,
                                             f"P{ro}_{k}")
                    for ji, (j, blk) in enumerate(
                            [(0, c8), (3, cm), (5, am), (7, wm)]):
                        wt = stream.tile([128, D], BF16, tag="w1w",
                                         name=f"wP{ro}_{k}_{j}")
                        nc.sync.dma_start(
                            out=wt[:, :],
                            in_=w1T[j * D + k * 128: j * D + (k + 1) * 128, :])
                        for m in range(KC):
                            nc.tensor.matmul(
                                zps[m][:, :], wt[:, m * 128:(m + 1) * 128],
                                blk[:, :], start=(k == 0 and ji == 0),
                                stop=(k == KC - 1 and ji == 3))
                        yield
                for m in range(KC):
                    nc.scalar.activation(
                        p_sb[:, m * ROWS + ro: m * ROWS + ro + rl],
                        zps[m][:, :], AF.Copy)
                    if m % 4 == 3:
                        yield
                ps.release()

        def z_g_phase(vecf_sb, wbv_sb, qc_sb, tagsfx, spans):
            """Z-delta + P + qc -> tanh -> W2 -> sigmoid -> g_sb.
            Generator; yields between sub-chunks."""
            for ro, rl in spans:
                ps = tc.alloc_tile_pool(name=f"zps{tagsfx}{ro}", bufs=1,
                                        space="PSUM")
                g1s = [scratch.tile([128, rl], BF16, tag=f"g1_{m}", bufs=1,
                                    name=f"g1_{tagsfx}{ro}_{m}")
                       for m in range(KC)]
                zp2 = [ps.tile([128, 2 * rl], F32, tag=f"zp{m2}",
                               name=f"zd{tagsfx}{ro}_{m2}")
                       for m2 in range(KC // 2)]
                zps = [zp2[m // 2][:, (m % 2) * rl:(m % 2 + 1) * rl]
                       for m in range(KC)]
                for k in range(KC):
                    cm, am, wm = make_blocks(k, ro, rl, vecf_sb, wbv_sb,
                                             f"D{tagsfx}{ro}_{k}")
                    for ji, (j, blk) in enumerate([(4, cm), (6, am), (8, wm)]):
                        wt = stream.tile([128, D], BF16, tag="w1w",
                                         name=f"wD{tagsfx}{ro}_{k}_{j}")
                        nc.sync.dma_start(
                            out=wt[:, :],
                            in_=w1T[j * D + k * 128: j * D + (k + 1) * 128, :])
                        for m in range(KC):
                            nc.tensor.matmul(
                                zps[m][:, :], wt[:, m * 128:(m + 1) * 128],
                                blk[:, :], start=(k == 0 and ji == 0),
                                stop=(k == KC - 1 and ji == 2))
                        yield
                for m in range(KC):
                    t1 = scratch.tile([128, rl], F32, tag="t1", bufs=2,
                                      name=f"t1_{tagsfx}{ro}_{m}")
                    nc.vector.scalar_tensor_tensor(
                        t1[:, :].rearrange("p (t b) -> p t b", b=BL),
                        zps[m][:, :].rearrange("p (t b) -> p t b", b=BL),
                        1.0,
                        qc_sb[:, m * BL:(m + 1) * BL].unsqueeze(1).broadcast_to(
                            [128, rl // BL, BL]),
                        ALU.mult, ALU.add)
                    t2 = scratch.tile([128, rl], F32, tag="t2", bufs=2,
                                      name=f"t2_{tagsfx}{ro}_{m}")
                    nc.vector.tensor_tensor(
                        t2[:, :], t1[:, :],
                        p_sb[:, m * ROWS + ro: m * ROWS + ro + rl],
                        ALU.add)
                    nc.scalar.activation(g1s[m][:, :], t2[:, :], AF.Tanh)
                    if m % 2 == 1:
                        yield
                # W2 phase (same pool tags: slots reuse, deps tracked)
                gp2_ = [ps.tile([128, 2 * rl], F32, tag=f"zp{m2}",
                                name=f"gw{tagsfx}{ro}_{m2}")
                        for m2 in range(KC // 2)]
                gps = [gp2_[m // 2][:, (m % 2) * rl:(m % 2 + 1) * rl]
                       for m in range(KC)]
                for k in range(KC):
                    wt = stream.tile([128, D], BF16, tag="w1w",
                                     name=f"w2_{tagsfx}{ro}_{k}")
                    nc.sync.dma_start(
                        out=wt[:, :], in_=w2T[k * 128:(k + 1) * 128, :])
                    for m in range(KC):
                        nc.tensor.matmul(
                            gps[m][:, :], wt[:, m * 128:(m + 1) * 128],
                            g1s[k][:, :],
                            start=(k == 0), stop=(k == KC - 1))
                    yield
                for m in range(KC):
                    nc.scalar.activation(
                        g_sb[:, m * ROWS + ro: m * ROWS + ro + rl],
                        gps[m][:, :], AF.Sigmoid, bias=bG_sb[:, m:m + 1])
                    if m % 4 == 3:
                        yield
                ps.release()

        def gi_att_phase(spans):
            """gi_att = c @ att_Wih.T + bias, for all 24 out-chunks.
            Generator; yields between sub-chunks. Span-major so scan row
            ranges complete early."""
            for ro, rl in spans:
                ps = tc.alloc_tile_pool(name=f"gips{ro}", bufs=1,
                                        space="PSUM")
                for mg in range(6):
                    pts = [ps.tile([128, rl], F32, tag=f"gp{mi}",
                                   name=f"gip{ro}_{mg}_{mi}")
                           for mi in range(4)]
                    for k in range(KC):
                        wt = stream.tile([128, 512], BF16, tag="wihw",
                                         name=f"wih{ro}_{mg}_{k}")
                        nc.sync.dma_start(
                            out=wt[:, :],
                            in_=aWihT[k * 128:(k + 1) * 128,
                                      mg * 512:(mg + 1) * 512])
                        c8b = cast_c(k, ro, rl)
                        for mi in range(4):
                            nc.tensor.matmul(
                                pts[mi][:, :],
                                wt[:, mi * 128:(mi + 1) * 128],
                                c8b[:, :],
                                start=(k == 0), stop=(k == KC - 1))
                        yield
                    for mi in range(4):
                        m = mg * 4 + mi
                        nc.scalar.activation(
                            gi_sb[:, m * ROWS + ro: m * ROWS + ro + rl],
                            pts[mi][:, :], AF.Identity,
                            bias=bA_sb[:, m:m + 1])
                    yield
                ps.release()

        def scan(nsteps, sfx, feeder=None, pump=0):
            """Attention-GRU scan; returns final h (bf16, [128, KC*BL]).

            Critical-path structure per step:
              - psum tiles for r / z / n are preloaded with 32*gi (gi_z
                negated on the host) one step ahead, so the matmuls
                accumulate straight onto the bias term and the sigmoids
                read PSUM directly: rs = sigmoid(pr/32), zs' = 1 - z =
                sigmoid(pz/32).
              - r-matmuls are emitted first so rs overlaps the z/n
                matmuls; the z-side products (gp = zs'*G, vv = 1-gp,
                w = vv*h) run on GPSIMD off the critical path.
              - chain: mm_r -> rs -> n1 -> n2 -> tanh -> u -> h'.
            """
            ps = tc.alloc_tile_pool(name="scanps", bufs=1, space="PSUM")
            nb_ = KC * BL
            h8 = scratch.tile([128, nb_], BF16, tag="h8", name=f"h8{sfx}")
            nc.vector.memset(h8[:, :], 0.0)
            h = h8
            gi_v = gi_sb[:, :].rearrange(
                "p (m t b) -> p m t b", m=3 * KC, b=BL)
            g_v = g_sb[:, :].rearrange("p (m t b) -> p m t b", m=KC, b=BL)

            def pr_pz_tiles(t):
                pr = ps.tile([128, nb_], F32, tag="ghpr",
                             name=f"ghpr{sfx}_{t}")
                pz = ps.tile([128, nb_], F32, tag="ghpz",
                             name=f"ghpz{sfx}_{t}")
                return pr, pz

            def preload(t, pr, pz):
                # psum <- 32*gi_r / 32*(-gi_z)  (z negation folded on host)
                nc.scalar.activation(
                    pr[:, :].rearrange("p (m b) -> p m b", b=BL),
                    gi_v[:, 0:KC, t, :], AF.Copy, scale=S_WHH)
                nc.gpsimd.tensor_scalar(
                    pz[:, :].rearrange("p (m b) -> p m b", b=BL),
                    gi_v[:, KC:2 * KC, t, :], S_WHH, None, ALU.mult)

            def burst(pt, moff, start):
                for m in range(KC):
                    mm = moff + m
                    for k in range(KC):
                        nc.tensor.matmul(
                            pt[:, m * BL:(m + 1) * BL],
                            whh_sb[:, k * 3 * D + mm * 128:
                                   k * 3 * D + (mm + 1) * 128],
                            h8[:, k * BL:(k + 1) * BL],
                            start=(start and k == 0 and m == 0),
                            stop=(k == KC - 1 and m == KC - 1),
                            skip_group_check=not start)

            pr, pz = pr_pz_tiles(0)
            preload(0, pr, pz)
            for t in range(nsteps):
                pn = ps.tile([128, nb_], F32, tag="ghpn",
                             name=f"ghpn{sfx}_{t}")
                burst(pr, 0, start=False)
                rs = scratch.tile([128, nb_], F32, tag="rs",
                                  name=f"rs{sfx}_{t}")
                nc.scalar.activation(rs[:, :], pr[:, :], AF.Sigmoid,
                                     scale=1.0 / S_WHH)
                burst(pz, KC, start=False)
                burst(pn, 2 * KC, start=True)
                # zs' = 1 - z = sigmoid(pz/32) (z-path negated on host)
                zs = scratch.tile([128, nb_], F32, tag="zs",
                                  name=f"zs{sfx}_{t}")
                nc.scalar.activation(zs[:, :], pz[:, :], AF.Sigmoid,
                                     scale=1.0 / S_WHH)
                # n path (critical): n1 = rs * pn/32 ; n2 = n1 + gi_n
                n1 = scratch.tile([128, nb_], F32, tag="n1",
                                  name=f"n1{sfx}_{t}")
                nc.vector.scalar_tensor_tensor(
                    n1[:, :], pn[:, :], 1.0 / S_WHH,
                    rs[:, :], ALU.mult, ALU.mult)
                n2 = scratch.tile([128, nb_], F32, tag="n2",
                                  name=f"n2{sfx}_{t}")
                nc.vector.tensor_tensor(
                    n2[:, :].rearrange("p (m b) -> p m b", b=BL),
                    n1[:, :].rearrange("p (m b) -> p m b", b=BL),
                    gi_v[:, 2 * KC:3 * KC, t, :], ALU.add)
                nt = scratch.tile([128, nb_], F32, tag="nt",
                                  name=f"nt{sfx}_{t}")
                nc.scalar.activation(nt[:, :], n2[:, :], AF.Tanh)
                # z side on GPSIMD (off critical path):
                # gp = zs'*G ; vv = 1-gp ; w = vv*h
                gp = scratch.tile([128, nb_], F32, tag="gp2",
                                  name=f"gp{sfx}_{t}")
                nc.gpsimd.tensor_tensor(
                    gp[:, :].rearrange("p (m b) -> p m b", b=BL),
                    zs[:, :].rearrange("p (m b) -> p m b", b=BL),
                    g_v[:, :, t, :], ALU.mult)
                vv = scratch.tile([128, nb_], F32, tag="vv",
                                  name=f"vv{sfx}_{t}")
                nc.gpsimd.tensor_scalar(
                    vv[:, :], gp[:, :], -1.0, 1.0, ALU.mult, ALU.add)
                w_ = scratch.tile([128, nb_], F32, tag="wv",
                                  name=f"wv{sfx}_{t}")
                nc.gpsimd.tensor_tensor(w_[:, :], vv[:, :], h[:, :], ALU.mult)
                # preload next step's psums while the tail computes
                if t + 1 < nsteps:
                    prn, pzn = pr_pz_tiles(t + 1)
                    preload(t + 1, prn, pzn)
                # h' = gp*nt + (1-gp)*h, carried in bf16
                u_ = scratch.tile([128, nb_], F32, tag="uv",
                                  name=f"uv{sfx}_{t}")
                nc.vector.tensor_tensor(u_[:, :], gp[:, :], nt[:, :], ALU.mult)
                hn = scratch.tile([128, nb_], BF16, tag="h8",
                                  name=f"h{sfx}_{t}")
                nc.vector.tensor_tensor(hn[:, :], u_[:, :], w_[:, :], ALU.add)
                h = hn
                h8 = hn
                if t + 1 < nsteps:
                    pr, pz = prn, pzn
                # pump precompute emission into this step's idle windows
                if feeder is not None:
                    for _ in range(pump):
                        if next(feeder, _DONE) is _DONE:
                            feeder = None
                            break
            ps.release()
            return h

        def mem_gru(e8_sb, m_sb, m8_sb, sfx):
            """m_new = GRUCell(e, m) with mem weights; feature-major."""
            ps = tc.alloc_tile_pool(name="memps", bufs=2, space="PSUM")
            nb = KC * BL
            gi_p = ps.tile([128, 3 * nb], F32, tag="memgh", name=f"mgi{sfx}")
            for k in range(KC):
                for g3 in range(3):
                    wt = stream.tile([128, D], BF16, tag="w1w",
                                     name=f"mw{sfx}_{k}_{g3}")
                    nc.sync.dma_start(
                        out=wt[:, :],
                        in_=mWihT[k * 128:(k + 1) * 128,
                                  g3 * D:(g3 + 1) * D])
                    for mm in range(KC):
                        m = g3 * KC + mm
                        nc.tensor.matmul(
                            gi_p[:, m * BL:(m + 1) * BL],
                            wt[:, mm * 128:(mm + 1) * 128],
                            e8_sb[:, k * BL:(k + 1) * BL],
                            start=(k == 0 and g3 == 0 and mm == 0),
                            stop=(k == KC - 1 and g3 == 2 and mm == KC - 1))
            gi_f = scratch.tile([128, 3 * nb], F32, tag="memgif",
                                name=f"mgif{sfx}")
            nc.vector.tensor_tensor(
                gi_f[:, :].rearrange("p (m b) -> p m b", b=BL),
                gi_p[:, :].rearrange("p (m b) -> p m b", b=BL),
                bM_sb[:, :].unsqueeze(2).broadcast_to([128, 3 * KC, BL]),
                ALU.add)
            gh_p = ps.tile([128, 3 * nb], F32, tag="memgh", name=f"mgh{sfx}")
            for k in range(KC):
                for g3 in range(3):
                    wt = stream.tile([128, D], BF16, tag="w1w",
                                     name=f"mwh{sfx}_{k}_{g3}")
                    nc.sync.dma_start(
                        out=wt[:, :],
                        in_=mWhhT[k * 128:(k + 1) * 128,
                                  g3 * D:(g3 + 1) * D])
                    for mm in range(KC):
                        m = g3 * KC + mm
                        nc.tensor.matmul(
                            gh_p[:, m * BL:(m + 1) * BL],
                            wt[:, mm * 128:(mm + 1) * 128],
                            m8_sb[:, k * BL:(k + 1) * BL],
                            start=(k == 0 and g3 == 0 and mm == 0),
                            stop=(k == KC - 1 and g3 == 2 and mm == KC - 1))
            rz = scratch.tile([128, 2 * nb], F32, tag="mrz", name=f"mrz{sfx}")
            nc.vector.tensor_tensor(
                rz[:, :], gi_f[:, 0:2 * nb], gh_p[:, 0:2 * nb], ALU.add)
            rzs = scratch.tile([128, 2 * nb], F32, tag="mrzs", name=f"mrzs{sfx}")
            nc.scalar.activation(rzs[:, :], rz[:, :], AF.Sigmoid)
            n1 = scratch.tile([128, nb], F32, tag="mn1", name=f"mn1{sfx}")
            nc.vector.tensor_tensor(
                n1[:, :], rzs[:, 0:nb], gh_p[:, 2 * nb:3 * nb], ALU.mult)
            n2 = scratch.tile([128, nb], F32, tag="mn2", name=f"mn2{sfx}")
            nc.vector.tensor_tensor(
                n2[:, :], n1[:, :], gi_f[:, 2 * nb:3 * nb], ALU.add)
            nt = scratch.tile([128, nb], F32, tag="mnt", name=f"mnt{sfx}")
            nc.scalar.activation(nt[:, :], n2[:, :], AF.Tanh)
            d1 = scratch.tile([128, nb], F32, tag="md1", name=f"md1{sfx}")
            nc.vector.tensor_tensor(d1[:, :], m_sb[:, :], nt[:, :],
                                    ALU.subtract)
            d2 = scratch.tile([128, nb], F32, tag="md2", name=f"md2{sfx}")
            nc.vector.tensor_tensor(d2[:, :], d1[:, :], rzs[:, nb:2 * nb],
                                    ALU.mult)
            mn = scratch.tile([128, nb], F32, tag="mnew", bufs=2,
                              name=f"mn{sfx}")
            nc.vector.tensor_tensor(mn[:, :], d2[:, :], nt[:, :], ALU.add)
            mn8 = scratch.tile([128, nb], BF16, tag="mnew8", bufs=2,
                               name=f"mn8{sfx}")
            nc.vector.tensor_copy(mn8[:, :], mn[:, :])
            ps.release()
            return mn, mn8

        # ================= episode 1 (m = q) =================
        # Head: rows 0..RB-1 (scan steps 0..RB/BL-1) emitted up front;
        # the remaining row-spans stream into the scan's idle PE windows.
        HEAD = [(0, 256)]
        TAILS = [(256, 256), (512, 256), (768, 256)]

        def drain(g):
            for _ in g:
                pass

        if 'gi' in phases:
            drain(gi_att_phase(HEAD))
        if 'P' in phases:
            drain(pass_P(qT_sb, wbq_sb, HEAD))
        if 'zg' in phases:
            drain(z_g_phase(qT_sb, wbq_sb, qc1_sb, "a", HEAD))
        feeder1 = None
        if all(p in phases for p in ('gi', 'P', 'zg')):
            feeder1 = itertools.chain(*[
                itertools.chain(
                    gi_att_phase([s]),
                    pass_P(qT_sb, wbq_sb, [s]),
                    z_g_phase(qT_sb, wbq_sb, qc1_sb, f"a{si}", [s]))
                for si, s in enumerate(TAILS)])
        if 'scan' in phases:
            h1 = scan(t_steps, "a", feeder=feeder1, pump=6)
        else:
            h1 = scratch.tile([128, KC * BL], BF16, tag="h8", name="hstub_a")
            nc.vector.memset(h1[:, :], 0.0)
        if feeder1 is not None:
            drain(feeder1)
        m1, m1_8 = mem_gru(h1, qT_sb, qb_sb, "a")

        # ================= episode 2 (m = m1) =================
        wbm_sb = pool.tile([128, KC * BL], F32, tag="wbm")
        if 'small' in phases:
            small_matmul(wbT, m1_8, wbm_sb)
        else:
            nc.vector.memset(wbm_sb[:, :], 0.0)

        class _W1Slice:
            """View of w1T rows [off, off+D) as a [D, D] dram tensor."""
            def __init__(self, off):
                self.off = off
            def __getitem__(self, idx):
                ksl, msl = idx
                return w1T[self.off + ksl.start: self.off + ksl.stop, msl]

        qc2a = pool.tile([128, KC * BL], F32, tag="qc2a")
        qc2 = pool.tile([128, KC * BL], F32, tag="qc2")
        if 'small' in phases:
            small_matmul(_W1Slice(1 * D), m1_8, qc2a)
            small_matmul(_W1Slice(2 * D), qb_sb, qc2, bias=bZ_sb,
                         accum_from=qc2a)
        else:
            nc.vector.memset(qc2[:, :], 0.0)

        feeder2 = None
        if 'zg' in phases:
            drain(z_g_phase(m1, wbm_sb, qc2, "b", [(0, 256)]))
            feeder2 = z_g_phase(m1, wbm_sb, qc2, "b2",
                                [(256, 256), (512, 256), (768, 256)])
        if 'scan' in phases:
            h2 = scan(t_steps, "b", feeder=feeder2, pump=3)
        else:
            h2 = scratch.tile([128, KC * BL], BF16, tag="h8", name="hstub_b")
            nc.vector.memset(h2[:, :], 0.0)
        if feeder2 is not None:
            drain(feeder2)
        m2, _ = mem_gru(h2, m1, m1_8, "b")

        nc.sync.dma_start(out=out[:, :], in_=m2[:, :])

        for p in (scratch, stream, pool):
            p.release()

    if split_waits:
        _split_multiwait(nc)
    return nc


_cache = {}


def _get_nc(t_steps=T):
    if t_steps not in _cache:
        _cache[t_steps] = _build(t_steps)
    return _cache[t_steps]


def _prep_inputs(c, q, Wb_w, W1_w, W1_b, W2_w, W2_b,
                 mem_Wih, mem_Whh, mem_bih, mem_bhh,
                 att_Wih, att_Whh, att_bih, att_bhh):
    """Host-side: transpose/cast/shard everything into per-core in_maps."""
    f32 = np.float32
    c = np.asarray(c, f32); q = np.asarray(q, f32)
    W1j = [np.asarray(W1_w[:, j * D:(j + 1) * D], f32) for j in range(9)]

    def _negz():  # [1, 3D] row: negate the z-gate columns of att weights
        s = np.ones((1, 3 * D), f32)
        s[:, D:2 * D] = -1.0
        return s

    def fold_bias(v):  # [D] -> [128, KC] (p, m)
        return np.ascontiguousarray(
            np.asarray(v, f32).reshape(KC, 128).T)

    def fold_bias3(bih, bhh, negate_z=False):  # [3D] -> [128, 3KC]
        v = np.asarray(bih, f32).copy()
        bhh = np.asarray(bhh, f32)
        v[:2 * D] += bhh[:2 * D]
        if negate_z:  # z-gate negated so the kernel gets 1-z from sigmoid
            v[D:2 * D] *= -1.0
        return np.ascontiguousarray(v.reshape(3 * KC, 128).T)

    shared = {
        "w1T": np.ascontiguousarray(np.asarray(W1_w, f32).T).astype(bf16_np),
        "w12T": np.ascontiguousarray((W1j[1] + W1j[2]).T).astype(bf16_np),
        "wbT": np.ascontiguousarray(np.asarray(Wb_w, f32).T).astype(bf16_np),
        "w2T": np.ascontiguousarray(np.asarray(W2_w, f32).T).astype(bf16_np),
        "aWihT": np.ascontiguousarray(
            np.asarray(att_Wih, f32).T * _negz()).astype(bf16_np),
        "aWhhT8": (np.ascontiguousarray(np.asarray(att_Whh, f32).T * _negz())
                   * S_WHH).astype(fp8_np),
        "mWihT": np.ascontiguousarray(np.asarray(mem_Wih, f32).T).astype(bf16_np),
        "mWhhT": np.ascontiguousarray(np.asarray(mem_Whh, f32).T).astype(bf16_np),
        "bZ": fold_bias(W1_b),
        "bG": fold_bias(W2_b),
        "bA": fold_bias3(att_bih, att_bhh, negate_z=True),
        "bM": fold_bias3(mem_bih, mem_bhh),
    }
    assert not np.any(np.asarray(att_bhh, f32)[2 * D:]), \
        "nonzero att_bhh n-gate bias not supported by this kernel build"
    assert not np.any(np.asarray(mem_bhh, f32)[2 * D:]), \
        "nonzero mem_bhh n-gate bias not supported by this kernel build"

    in_maps = []
    for ci in range(NCORES):
        s = ci * BL
        csh = c[:, s:s + BL, :].reshape(ROWS, D)
        qsh = q[s:s + BL, :]
        im = dict(shared)
        im["cT"] = np.ascontiguousarray(csh.T)
        im["qT"] = np.ascontiguousarray(
            qsh.reshape(BL, KC, 128).transpose(2, 1, 0).reshape(128, KC * BL))
        in_maps.append(im)
    return in_maps


def _unshard(results):
    m = np.empty((B, D), np.float32)
    for ci in range(NCORES):
        o = results[ci]["out"]  # [128, KC*BL]: [p, (k, b)]
        m[ci * BL:(ci + 1) * BL] = (
            o.reshape(128, KC, BL).transpose(2, 1, 0).reshape(BL, D))
    return m


def run_device(in_maps, trace=False):
    nc = _get_nc()
    res = run_bass_kernel_spmd(nc, in_maps, list(range(NCORES)), trace=trace)
    return res


def kernel(**inputs) -> np.ndarray:
    in_maps = _prep_inputs(**inputs)
    res = run_device(in_maps)
    return _unshard(res.results)


if __name__ == "__main__":
    np.random.seed(0)
    pass



# revision 39
# speedup vs baseline: 1.0759x; 1.0759x over previous
"""Trainium2 Bass kernel for nn_EpisodicMemoryModule.

Strategy
--------
Math restructure: inside each episode's scan, the gate chain
z -> g1 -> g depends only on (c_t, m, q) -- never on h. So G[t] for all
timesteps is precomputed with large batched matmuls, as is the
x-dependent half of the attention GRU (gi_att = c @ att_Wih.T, which is
even episode-invariant). The only sequential work left per scan step is
gh = h @ att_Whh.T plus elementwise gates.

Sharding: data-parallel over batch B=64 across 8 cores (8 rows/core),
no inter-core communication. All weights are pre-transposed on the host
into contraction-major layout (features on partitions) and pre-cast:
bf16 for everything except att_Whh.T which is fp8e4m3 scaled by 32
(the GRU recurrence is contractive; end-to-end rel err ~1e-3).

Within a core everything is feature-major ([d partitions, (t,b) free]),
which keeps the per-step gate elementwise work on fully-occupied
128-partition tiles.
"""

import itertools
import sys

sys.path.insert(0, "/opt/trn_rl_repo")

_DONE = object()

import numpy as np
import ml_dtypes

import concourse.bass as bass
import concourse.mybir as mybir
from concourse.bass_utils import run_bass_kernel_spmd
from concourse.tile import TileContext
import bass_rust
from bass_rust import ScopedClock

T, B, D = 128, 64, 1024
NCORES = 8
BL = B // NCORES          # 8 batch rows per core
ROWS = T * BL             # 1024 rows per core
RB = 512                  # row-block for precompute matmuls
NRB = ROWS // RB          # 2
KC = D // 128             # 8 contraction chunks
S_WHH = 32.0              # fp8 scale for att_Whh

F32 = mybir.dt.float32
BF16 = mybir.dt.bfloat16
FP8 = mybir.dt.float8e4
AF = mybir.ActivationFunctionType
ALU = mybir.AluOpType

bf16_np = ml_dtypes.bfloat16
fp8_np = ml_dtypes.float8_e4m3fn


class _TC(TileContext):
    """TileContext whose final drain splits multi-sem waits (the walrus in
    this environment accepts only one sync wait per instruction)."""

    def _drain_and_barrier(self, tick_clock, wait_clock):
        drain_inst = self.nc.sync.drain()
        wait_clock.add_sem_waits(
            drain_inst.ins, ScopedClock({None: tick_clock.global_clock})
        )
        si = drain_inst.ins.sync_info
        if si is not None and si.on_wait and len(si.on_wait) > 1:
            waits = list(si.on_wait)
            drain_inst.ins.sync_info = bass_rust.SyncInfo(
                on_wait=[waits[0]], on_update=list(si.on_update or [])
            )
            for w in waits[1:]:
                d = self.nc.sync.drain()
                d.ins.sync_info = bass_rust.SyncInfo(on_wait=[w], on_update=[])
        self.nc.all_engine_barrier()
        assert self.sems is not None
        popped = self.nc._tile_sem_poison_stack.pop()
        assert popped is self._sem_poison
        self.nc.clear_and_free_semaphores(list(self.sems.allocated().values()))
        self.nc.all_engine_barrier()


def _split_multiwait(nc):
    """Split >1-wait instructions into single-wait NoOps + instruction."""
    nfix = 0
    for f in nc.m.functions:
        for bb in f.blocks:
            insts = list(bb.instructions)
            out = []
            changed = False
            for inst in insts:
                si = inst.sync_info
                if si and si.on_wait and len(si.on_wait) > 1:
                    waits = list(si.on_wait)
                    for i, w in enumerate(waits[:-1]):
                        nop = mybir.InstNoOp(
                            name=f"I-waitfix-{nfix}-{i}", ins=[], outs=[]
                        )
                        nop.engine = inst.engine
                        nop.sync_info = bass_rust.SyncInfo(on_wait=[w], on_update=[])
                        out.append(nop)
                    inst.sync_info = bass_rust.SyncInfo(
                        on_wait=[waits[-1]], on_update=list(si.on_update or [])
                    )
                    nfix += 1
                    changed = True
                out.append(inst)
            if changed:
                bb.instructions = out
    return nfix


def _build(t_steps=T, split_waits=True, phases=('gi', 'P', 'zg', 'scan', 'mem', 'small')):
    """Build the per-core Bass module (SPMD; every core runs the same
    program on its own batch shard)."""
    nc = bass.Bass()
    P = nc.declare_dram_parameter

    # Per-core activations (feature-major) and vectors.
    cT = P("cT", [D, ROWS], F32, isOutput=False)            # c.T shard
    qT = P("qT", [128, KC * BL], F32, isOutput=False)       # q.T folded (p,(k,b))
    # Weights, contraction-major (in-features on partitions).
    w1T = P("w1T", [9 * D, D], BF16, isOutput=False)        # W1.T
    w12T = P("w12T", [D, D], BF16, isOutput=False)          # (W1_1+W1_2).T
    wbT = P("wbT", [D, D], BF16, isOutput=False)            # Wb.T
    w2T = P("w2T", [D, D], BF16, isOutput=False)            # W2.T
    aWihT = P("aWihT", [D, 3 * D], BF16, isOutput=False)    # att_Wih.T
    aWhhT8 = P("aWhhT8", [D, 3 * D], FP8, isOutput=False)   # att_Whh.T * 32 fp8
    mWihT = P("mWihT", [D, 3 * D], BF16, isOutput=False)
    mWhhT = P("mWhhT", [D, 3 * D], BF16, isOutput=False)
    bZ = P("bZ", [128, KC], F32, isOutput=False)            # W1_b per (p, m)
    bG = P("bG", [128, KC], F32, isOutput=False)            # W2_b
    bA = P("bA", [128, 3 * KC], F32, isOutput=False)        # att bih(+bhh for r,z)
    bM = P("bM", [128, 3 * KC], F32, isOutput=False)        # mem bih(+bhh for r,z)
    out = P("out", [128, KC * BL], F32, isOutput=True)      # m2.T folded

    with _TC(nc) as tc:
        pool = tc.alloc_tile_pool(name="res", bufs=1)
        stream = tc.alloc_tile_pool(name="stream", bufs=6)
        scratch = tc.alloc_tile_pool(name="scratch", bufs=3)

        # ---- resident loads -------------------------------------------------
        cT_sb = pool.tile([128, KC * ROWS], F32, tag="cT")     # 32 KB/par
        for k in range(KC):
            nc.sync.dma_start(
                out=cT_sb[:, k * ROWS:(k + 1) * ROWS],
                in_=cT[k * 128:(k + 1) * 128, :],
            )
        qT_sb = pool.tile([128, KC * BL], F32, tag="qT")
        nc.sync.dma_start(out=qT_sb[:, :], in_=qT[:, :])
        whh_sb = pool.tile([128, KC * 3 * D], FP8, tag="whh")  # 24 KB/par
        for k in range(KC):
            nc.sync.dma_start(
                out=whh_sb[:, k * 3 * D:(k + 1) * 3 * D],
                in_=aWhhT8[k * 128:(k + 1) * 128, :],
            )
        bZ_sb = pool.tile([128, KC], F32, tag="bZ")
        nc.sync.dma_start(out=bZ_sb[:, :], in_=bZ[:, :])
        bG_sb = pool.tile([128, KC], F32, tag="bG")
        nc.sync.dma_start(out=bG_sb[:, :], in_=bG[:, :])
        bA_sb = pool.tile([128, 3 * KC], F32, tag="bA")
        nc.sync.dma_start(out=bA_sb[:, :], in_=bA[:, :])
        bM_sb = pool.tile([128, 3 * KC], F32, tag="bM")
        nc.sync.dma_start(out=bM_sb[:, :], in_=bM[:, :])

        qb_sb = pool.tile([128, KC * BL], BF16, tag="qb")
        nc.vector.tensor_copy(qb_sb[:, :], qT_sb[:, :])

        gi_sb = pool.tile([128, 3 * KC * ROWS], BF16, tag="gi")  # 48 KB/par
        p_sb = pool.tile([128, KC * ROWS], BF16, tag="P")        # 16 KB/par
        g_sb = pool.tile([128, KC * ROWS], BF16, tag="G")        # 16 KB/par

        def small_matmul(wT_dram, vec_sb, out_sb, bias=None, accum_from=None,
                         tagp="smallp"):
            """out.T[dout, BL] = W @ vec  (all feature-major [128, KC*BL]).
            One psum bank, k-outer single accumulation group, chunked weight
            DMA, fused bias/accum adds."""
            ps = tc.alloc_tile_pool(name="smallps", bufs=1, space="PSUM")
            pt = ps.tile([128, KC * BL], F32, tag=tagp, name=f"spt_{tagp}")
            for k in range(KC):
                wt = stream.tile([128, D], BF16, tag="w1w", name=f"sw_{tagp}{k}")
                nc.sync.dma_start(
                    out=wt[:, :], in_=wT_dram[k * 128:(k + 1) * 128, :])
                for m in range(KC):
                    nc.tensor.matmul(
                        pt[:, m * BL:(m + 1) * BL],
                        wt[:, m * 128:(m + 1) * 128],
                        vec_sb[:, k * BL:(k + 1) * BL],
                        start=(k == 0 and m == 0),
                        stop=(k == KC - 1 and m == KC - 1),
                    )
            src0 = pt[:, :]
            if bias is not None:
                nc.vector.tensor_tensor(
                    out_sb[:, :].rearrange("p (m b) -> p m b", b=BL),
                    src0.rearrange("p (m b) -> p m b", b=BL),
                    bias[:, :].unsqueeze(2).broadcast_to([128, KC, BL]),
                    ALU.add)
                src0 = out_sb[:, :]
            if accum_from is not None:
                nc.vector.tensor_tensor(
                    out_sb[:, :], src0, accum_from[:, :], ALU.add)
            elif bias is None:
                nc.vector.tensor_copy(out_sb[:, :], src0)
            ps.release()

        # ---- Wbq = q @ Wb.T ; qc1 = q @ (W1_1+W1_2).T + W1_b ---------------
        wbq_sb = pool.tile([128, KC * BL], F32, tag="wbq")
        qc1_sb = pool.tile([128, KC * BL], F32, tag="qc1")
        if 'small' in phases:
            small_matmul(wbT, qb_sb, wbq_sb)
            small_matmul(w12T, qb_sb, qc1_sb, bias=bZ_sb)
        else:
            nc.vector.memset(wbq_sb[:, :], 0.0)
            nc.vector.memset(qc1_sb[:, :], 0.0)

        def cast_c(k, ro, rl):
            c8 = scratch.tile([128, rl], BF16, tag="c8", bufs=2,
                              name=f"c8_{k}_{ro}")
            nc.vector.tensor_copy(
                c8[:, :], cT_sb[:, k * ROWS + ro: k * ROWS + ro + rl])
            return c8

        def make_blocks(k, ro, rl, vecf_sb, wbv_sb, tagsfx):
            """m-dependent blocks (c*v, |c-v|, c*Wbv) for chunk k, rows
            [ro, ro+rl); bf16 [128, rl] each."""
            cslice = cT_sb[:, k * ROWS + ro: k * ROWS + ro + rl]
            cview = cslice.rearrange("p (t b) -> p t b", b=BL)
            vv = vecf_sb[:, k * BL:(k + 1) * BL].unsqueeze(1).broadcast_to(
                [128, rl // BL, BL])
            wv = wbv_sb[:, k * BL:(k + 1) * BL].unsqueeze(1).broadcast_to(
                [128, rl // BL, BL])
            cm = scratch.tile([128, rl], BF16, tag="blk_cm", bufs=2,
                              name=f"cm{tagsfx}")
            nc.vector.tensor_tensor(
                cm[:, :].rearrange("p (t b) -> p t b", b=BL), cview, vv, ALU.mult)
            tmp = scratch.tile([128, rl], F32, tag="blk_tmp", bufs=2,
                               name=f"bt{tagsfx}")
            nc.vector.tensor_tensor(
                tmp[:, :].rearrange("p (t b) -> p t b", b=BL), cview, vv,
                ALU.subtract)
            am = scratch.tile([128, rl], BF16, tag="blk_am", bufs=2,
                              name=f"am{tagsfx}")
            nc.scalar.activation(am[:, :], tmp[:, :], AF.Abs)
            wm = scratch.tile([128, rl], BF16, tag="blk_wm", bufs=2,
                              name=f"wm{tagsfx}")
            nc.vector.tensor_tensor(
                wm[:, :].rearrange("p (t b) -> p t b", b=BL), cview, wv, ALU.mult)
            return cm, am, wm

        def pass_P(vecf_sb, wbv_sb, spans):
            """Shared Z part: P = c@W1_0.T + (c*q)@W1_3.T + |c-q|@W1_5.T
            + (c*Wbq)@W1_7.T  -> p_sb (bf16). Generator: yields between
            sub-chunks so callers can interleave emission with the scan."""
            for si, (ro, rl) in enumerate(spans):
                ps = tc.alloc_tile_pool(name=f"Pps{ro}", bufs=1, space="PSUM")
                zp2 = [ps.tile([128, 2 * rl], F32, tag=f"zp{m2}",
                               name=f"pp{ro}_{m2}")
                       for m2 in range(KC // 2)]
                zps = [zp2[m // 2][:, (m % 2) * rl:(m % 2 + 1) * rl]
                       for m in range(KC)]
                for k in range(KC):
                    c8 = cast_c(k, ro, rl)
                    cm, am, wm = make_blocks(k, ro, rl, vecf_sb, wbv_sb,
                                             f"P{ro}_{k}")
                    for ji, (j, blk) in enumerate(
                            [(0, c8), (3, cm), (5, am), (7, wm)]):
                        wt = stream.tile([128, D], BF16, tag="w1w",
                                         name=f"wP{ro}_{k}_{j}")
                        nc.sync.dma_start(
                            out=wt[:, :],
                            in_=w1T[j * D + k * 128: j * D + (k + 1) * 128, :])
                        for m in range(KC):
                            nc.tensor.matmul(
                                zps[m][:, :], wt[:, m * 128:(m + 1) * 128],
                                blk[:, :], start=(k == 0 and ji == 0),
                                stop=(k == KC - 1 and ji == 3))
                        yield
                for m in range(KC):
                    nc.scalar.activation(
                        p_sb[:, m * ROWS + ro: m * ROWS + ro + rl],
                        zps[m][:, :], AF.Copy)
                    if m % 4 == 3:
                        yield
                ps.release()

        def z_g_phase(vecf_sb, wbv_sb, qc_sb, tagsfx, spans):
            """Z-delta + P + qc -> tanh -> W2 -> sigmoid -> g_sb.
            Generator; yields between sub-chunks."""
            for ro, rl in spans:
                ps = tc.alloc_tile_pool(name=f"zps{tagsfx}{ro}", bufs=1,
                                        space="PSUM")
                g1s = [scratch.tile([128, rl], BF16, tag=f"g1_{m}", bufs=1,
                                    name=f"g1_{tagsfx}{ro}_{m}")
                       for m in range(KC)]
                zp2 = [ps.tile([128, 2 * rl], F32, tag=f"zp{m2}",
                               name=f"zd{tagsfx}{ro}_{m2}")
                       for m2 in range(KC // 2)]
                zps = [zp2[m // 2][:, (m % 2) * rl:(m % 2 + 1) * rl]
                       for m in range(KC)]
                for k in range(KC):
                    cm, am, wm = make_blocks(k, ro, rl, vecf_sb, wbv_sb,
                                             f"D{tagsfx}{ro}_{k}")
                    for ji, (j, blk) in enumerate([(4, cm), (6, am), (8, wm)]):
                        wt = stream.tile([128, D], BF16, tag="w1w",
                                         name=f"wD{tagsfx}{ro}_{k}_{j}")
                        nc.sync.dma_start(
                            out=wt[:, :],
                            in_=w1T[j * D + k * 128: j * D + (k + 1) * 128, :])
                        for m in range(KC):
                            nc.tensor.matmul(
                                zps[m][:, :], wt[:, m * 128:(m + 1) * 128],
                                blk[:, :], start=(k == 0 and ji == 0),
                                stop=(k == KC - 1 and ji == 2))
                        yield
                for m in range(KC):
                    t1 = scratch.tile([128, rl], F32, tag="t1", bufs=2,
                                      name=f"t1_{tagsfx}{ro}_{m}")
                    nc.vector.scalar_tensor_tensor(
                        t1[:, :].rearrange("p (t b) -> p t b", b=BL),
                        zps[m][:, :].rearrange("p (t b) -> p t b", b=BL),
                        1.0,
                        qc_sb[:, m * BL:(m + 1) * BL].unsqueeze(1).broadcast_to(
                            [128, rl // BL, BL]),
                        ALU.mult, ALU.add)
                    t2 = scratch.tile([128, rl], F32, tag="t2", bufs=2,
                                      name=f"t2_{tagsfx}{ro}_{m}")
                    nc.vector.tensor_tensor(
                        t2[:, :], t1[:, :],
                        p_sb[:, m * ROWS + ro: m * ROWS + ro + rl],
                        ALU.add)
                    nc.scalar.activation(g1s[m][:, :], t2[:, :], AF.Tanh)
                    if m % 2 == 1:
                        yield
                # W2 phase (same pool tags: slots reuse, deps tracked)
                gp2_ = [ps.tile([128, 2 * rl], F32, tag=f"zp{m2}",
                                name=f"gw{tagsfx}{ro}_{m2}")
                        for m2 in range(KC // 2)]
                gps = [gp2_[m // 2][:, (m % 2) * rl:(m % 2 + 1) * rl]
                       for m in range(KC)]
                for k in range(KC):
                    wt = stream.tile([128, D], BF16, tag="w1w",
                                     name=f"w2_{tagsfx}{ro}_{k}")
                    nc.sync.dma_start(
                        out=wt[:, :], in_=w2T[k * 128:(k + 1) * 128, :])
                    for m in range(KC):
                        nc.tensor.matmul(
                            gps[m][:, :], wt[:, m * 128:(m + 1) * 128],
                            g1s[k][:, :],
                            start=(k == 0), stop=(k == KC - 1))
                    yield
                for m in range(KC):
                    nc.scalar.activation(
                        g_sb[:, m * ROWS + ro: m * ROWS + ro + rl],
                        gps[m][:, :], AF.Sigmoid, bias=bG_sb[:, m:m + 1])
                    if m % 4 == 3:
                        yield
                ps.release()

        def gi_att_phase(spans):
            """gi_att = c @ att_Wih.T + bias, for all 24 out-chunks.
            Generator; yields between sub-chunks. Span-major so scan row
            ranges complete early."""
            for ro, rl in spans:
                ps = tc.alloc_tile_pool(name=f"gips{ro}", bufs=1,
                                        space="PSUM")
                for mg in range(6):
                    pts = [ps.tile([128, rl], F32, tag=f"gp{mi}",
                                   name=f"gip{ro}_{mg}_{mi}")
                           for mi in range(4)]
                    for k in range(KC):
                        wt = stream.tile([128, 512], BF16, tag="wihw",
                                         name=f"wih{ro}_{mg}_{k}")
                        nc.sync.dma_start(
                            out=wt[:, :],
                            in_=aWihT[k * 128:(k + 1) * 128,
                                      mg * 512:(mg + 1) * 512])
                        c8b = cast_c(k, ro, rl)
                        for mi in range(4):
                            nc.tensor.matmul(
                                pts[mi][:, :],
                                wt[:, mi * 128:(mi + 1) * 128],
                                c8b[:, :],
                                start=(k == 0), stop=(k == KC - 1))
                        yield
                    for mi in range(4):
                        m = mg * 4 + mi
                        nc.scalar.activation(
                            gi_sb[:, m * ROWS + ro: m * ROWS + ro + rl],
                            pts[mi][:, :], AF.Identity,
                            bias=bA_sb[:, m:m + 1])
                    yield
                ps.release()

        def scan(nsteps, sfx, feeder=None, pump=0):
            """Attention-GRU scan; returns final h (bf16, [128, KC*BL]).

            Critical-path structure per step:
              - psum tiles for r / z / n are preloaded with 32*gi (gi_z
                negated on the host) one step ahead, so the matmuls
                accumulate straight onto the bias term and the sigmoids
                read PSUM directly: rs = sigmoid(pr/32), zs' = 1 - z =
                sigmoid(pz/32).
              - r-matmuls are emitted first so rs overlaps the z/n
                matmuls; the z-side products (gp = zs'*G, vv = 1-gp,
                w = vv*h) run on GPSIMD off the critical path.
              - chain: mm_r -> rs -> n1 -> n2 -> tanh -> u -> h'.
            """
            ps = tc.alloc_tile_pool(name="scanps", bufs=1, space="PSUM")
            nb_ = KC * BL
            h8 = scratch.tile([128, nb_], BF16, tag="h8", name=f"h8{sfx}")
            nc.vector.memset(h8[:, :], 0.0)
            h = h8
            gi_v = gi_sb[:, :].rearrange(
                "p (m t b) -> p m t b", m=3 * KC, b=BL)
            g_v = g_sb[:, :].rearrange("p (m t b) -> p m t b", m=KC, b=BL)

            def pr_pz_tiles(t):
                pr = ps.tile([128, nb_], F32, tag="ghpr",
                             name=f"ghpr{sfx}_{t}")
                pz = ps.tile([128, nb_], F32, tag="ghpz",
                             name=f"ghpz{sfx}_{t}")
                return pr, pz

            def preload(t, pr, pz):
                # psum <- 32*gi_r / 32*(-gi_z)  (z negation folded on host)
                nc.scalar.activation(
                    pr[:, :].rearrange("p (m b) -> p m b", b=BL),
                    gi_v[:, 0:KC, t, :], AF.Copy, scale=S_WHH)
                nc.gpsimd.tensor_scalar(
                    pz[:, :].rearrange("p (m b) -> p m b", b=BL),
                    gi_v[:, KC:2 * KC, t, :], S_WHH, None, ALU.mult)

            def burst(pt, moff, start):
                for m in range(KC):
                    mm = moff + m
                    for k in range(KC):
                        nc.tensor.matmul(
                            pt[:, m * BL:(m + 1) * BL],
                            whh_sb[:, k * 3 * D + mm * 128:
                                   k * 3 * D + (mm + 1) * 128],
                            h8[:, k * BL:(k + 1) * BL],
                            start=(start and k == 0 and m == 0),
                            stop=(k == KC - 1 and m == KC - 1),
                            skip_group_check=not start)

            pr, pz = pr_pz_tiles(0)
            preload(0, pr, pz)
            for t in range(nsteps):
                pn = ps.tile([128, nb_], F32, tag="ghpn",
                             name=f"ghpn{sfx}_{t}")
                burst(pr, 0, start=False)
                rs = scratch.tile([128, nb_], F32, tag="rs",
                                  name=f"rs{sfx}_{t}")
                nc.scalar.activation(rs[:, :], pr[:, :], AF.Sigmoid,
                                     scale=1.0 / S_WHH)
                burst(pz, KC, start=False)
                burst(pn, 2 * KC, start=True)
                # zs' = 1 - z = sigmoid(pz/32) (z-path negated on host)
                zs = scratch.tile([128, nb_], F32, tag="zs",
                                  name=f"zs{sfx}_{t}")
                nc.scalar.activation(zs[:, :], pz[:, :], AF.Sigmoid,
                                     scale=1.0 / S_WHH)
                # n path (critical): n1 = rs * pn/32 ; n2 = n1 + gi_n
                n1 = scratch.tile([128, nb_], F32, tag="n1",
                                  name=f"n1{sfx}_{t}")
                nc.vector.scalar_tensor_tensor(
                    n1[:, :], pn[:, :], 1.0 / S_WHH,
                    rs[:, :], ALU.mult, ALU.mult)
                n2 = scratch.tile([128, nb_], F32, tag="n2",
                                  name=f"n2{sfx}_{t}")
                nc.vector.tensor_tensor(
                    n2[:, :].rearrange("p (m b) -> p m b", b=BL),
                    n1[:, :].rearrange("p (m b) -> p m b", b=BL),
                    gi_v[:, 2 * KC:3 * KC, t, :], ALU.add)
                nt = scratch.tile([128, nb_], F32, tag="nt",
                                  name=f"nt{sfx}_{t}")
                nc.scalar.activation(nt[:, :], n2[:, :], AF.Tanh)
                # z side on GPSIMD (off critical path):
                # gp = zs'*G ; vv = 1-gp ; w = vv*h
                gp = scratch.tile([128, nb_], F32, tag="gp2",
                                  name=f"gp{sfx}_{t}")
                nc.gpsimd.tensor_tensor(
                    gp[:, :].rearrange("p (m b) -> p m b", b=BL),
                    zs[:, :].rearrange("p (m b) -> p m b", b=BL),
                    g_v[:, :, t, :], ALU.mult)
                vv = scratch.tile([128, nb_], F32, tag="vv",
                                  name=f"vv{sfx}_{t}")
                nc.gpsimd.tensor_scalar(
                    vv[:, :], gp[:, :], -1.0, 1.0, ALU.mult, ALU.add)
                w_ = scratch.tile([128, nb_], F32, tag="wv",
                                  name=f"wv{sfx}_{t}")
                nc.gpsimd.tensor_tensor(w_[:, :], vv[:, :], h[:, :], ALU.mult)
                # preload next step's psums while the tail computes
                if t + 1 < nsteps:
                    prn, pzn = pr_pz_tiles(t + 1)
                    preload(t + 1, prn, pzn)
                # h' = gp*nt + (1-gp)*h, carried in bf16
                u_ = scratch.tile([128, nb_], F32, tag="uv",
                                  name=f"uv{sfx}_{t}")
                nc.vector.tensor_tensor(u_[:, :], gp[:, :], nt[:, :], ALU.mult)
                hn = scratch.tile([128, nb_], BF16, tag="h8",
                                  name=f"h{sfx}_{t}")
                nc.vector.tensor_tensor(hn[:, :], u_[:, :], w_[:, :], ALU.add)
                h = hn
                h8 = hn
                if t + 1 < nsteps:
                    pr, pz = prn, pzn
                # pump precompute emission into this step's idle windows
                if feeder is not None:
                    for _ in range(pump):
                        if next(feeder, _DONE) is _DONE:
                            feeder = None
                            break
            ps.release()
            return h

        def mem_gru(e8_sb, m_sb, m8_sb, sfx):
            """m_new = GRUCell(e, m) with mem weights; feature-major."""
            ps = tc.alloc_tile_pool(name="memps", bufs=2, space="PSUM")
            nb = KC * BL
            gi_p = ps.tile([128, 3 * nb], F32, tag="memgh", name=f"mgi{sfx}")
            for k in range(KC):
                for g3 in range(3):
                    wt = stream.tile([128, D], BF16, tag="w1w",
                                     name=f"mw{sfx}_{k}_{g3}")
                    nc.sync.dma_start(
                        out=wt[:, :],
                        in_=mWihT[k * 128:(k + 1) * 128,
                                  g3 * D:(g3 + 1) * D])
                    for mm in range(KC):
                        m = g3 * KC + mm
                        nc.tensor.matmul(
                            gi_p[:, m * BL:(m + 1) * BL],
                            wt[:, mm * 128:(mm + 1) * 128],
                            e8_sb[:, k * BL:(k + 1) * BL],
                            start=(k == 0 and g3 == 0 and mm == 0),
                            stop=(k == KC - 1 and g3 == 2 and mm == KC - 1))
            gi_f = scratch.tile([128, 3 * nb], F32, tag="memgif",
                                name=f"mgif{sfx}")
            nc.vector.tensor_tensor(
                gi_f[:, :].rearrange("p (m b) -> p m b", b=BL),
                gi_p[:, :].rearrange("p (m b) -> p m b", b=BL),
                bM_sb[:, :].unsqueeze(2).broadcast_to([128, 3 * KC, BL]),
                ALU.add)
            gh_p = ps.tile([128, 3 * nb], F32, tag="memgh", name=f"mgh{sfx}")
            for k in range(KC):
                for g3 in range(3):
                    wt = stream.tile([128, D], BF16, tag="w1w",
                                     name=f"mwh{sfx}_{k}_{g3}")
                    nc.sync.dma_start(
                        out=wt[:, :],
                        in_=mWhhT[k * 128:(k + 1) * 128,
                                  g3 * D:(g3 + 1) * D])
                    for mm in range(KC):
                        m = g3 * KC + mm
                        nc.tensor.matmul(
                            gh_p[:, m * BL:(m + 1) * BL],
                            wt[:, mm * 128:(mm + 1) * 128],
                            m8_sb[:, k * BL:(k + 1) * BL],
                            start=(k == 0 and g3 == 0 and mm == 0),
                            stop=(k == KC - 1 and g3 == 2 and mm == KC - 1))
            rz = scratch.tile([128, 2 * nb], F32, tag="mrz", name=f"mrz{sfx}")
            nc.vector.tensor_tensor(
                rz[:, :], gi_f[:, 0:2 * nb], gh_p[:, 0:2 * nb], ALU.add)
            rzs = scratch.tile([128, 2 * nb], F32, tag="mrzs", name=f"mrzs{sfx}")
            nc.scalar.activation(rzs[:, :], rz[:, :], AF.Sigmoid)
            n1 = scratch.tile([128, nb], F32, tag="mn1", name=f"mn1{sfx}")
            nc.vector.tensor_tensor(
                n1[:, :], rzs[:, 0:nb], gh_p[:, 2 * nb:3 * nb], ALU.mult)
            n2 = scratch.tile([128, nb], F32, tag="mn2", name=f"mn2{sfx}")
            nc.vector.tensor_tensor(
                n2[:, :], n1[:, :], gi_f[:, 2 * nb:3 * nb], ALU.add)
            nt = scratch.tile([128, nb], F32, tag="mnt", name=f"mnt{sfx}")
            nc.scalar.activation(nt[:, :], n2[:, :], AF.Tanh)
            d1 = scratch.tile([128, nb], F32, tag="md1", name=f"md1{sfx}")
            nc.vector.tensor_tensor(d1[:, :], m_sb[:, :], nt[:, :],
                                    ALU.subtract)
            d2 = scratch.tile([128, nb], F32, tag="md2", name=f"md2{sfx}")
            nc.vector.tensor_tensor(d2[:, :], d1[:, :], rzs[:, nb:2 * nb],
                                    ALU.mult)
            mn = scratch.tile([128, nb], F32, tag="mnew", bufs=2,
                              name=f"mn{sfx}")
            nc.vector.tensor_tensor(mn[:, :], d2[:, :], nt[:, :], ALU.add)
            mn8 = scratch.tile([128, nb], BF16, tag="mnew8", bufs=2,
                               name=f"mn8{sfx}")
            nc.vector.tensor_copy(mn8[:, :], mn[:, :])
            ps.release()
            return mn, mn8

        # ================= episode 1 (m = q) =================
        # Head: rows 0..RB-1 (scan steps 0..RB/BL-1) emitted up front;
        # the remaining row-spans stream into the scan's idle PE windows.
        HEAD = [(0, 256)]
        TAILS = [(256, 256), (512, 256), (768, 256)]

        def drain(g):
            for _ in g:
                pass

        if 'gi' in phases:
            drain(gi_att_phase(HEAD))
        if 'P' in phases:
            drain(pass_P(qT_sb, wbq_sb, HEAD))
        if 'zg' in phases:
            drain(z_g_phase(qT_sb, wbq_sb, qc1_sb, "a", HEAD))
        feeder1 = None
        if all(p in phases for p in ('gi', 'P', 'zg')):
            feeder1 = itertools.chain(*[
                itertools.chain(
                    gi_att_phase([s]),
                    pass_P(qT_sb, wbq_sb, [s]),
                    z_g_phase(qT_sb, wbq_sb, qc1_sb, f"a{si}", [s]))
                for si, s in enumerate(TAILS)])
        if 'scan' in phases:
            h1 = scan(t_steps, "a", feeder=feeder1, pump=4)
        else:
            h1 = scratch.tile([128, KC * BL], BF16, tag="h8", name="hstub_a")
            nc.vector.memset(h1[:, :], 0.0)
        if feeder1 is not None:
            drain(feeder1)
        m1, m1_8 = mem_gru(h1, qT_sb, qb_sb, "a")

        # ================= episode 2 (m = m1) =================
        wbm_sb = pool.tile([128, KC * BL], F32, tag="wbm")
        if 'small' in phases:
            small_matmul(wbT, m1_8, wbm_sb)
        else:
            nc.vector.memset(wbm_sb[:, :], 0.0)

        class _W1Slice:
            """View of w1T rows [off, off+D) as a [D, D] dram tensor."""
            def __init__(self, off):
                self.off = off
            def __getitem__(self, idx):
                ksl, msl = idx
                return w1T[self.off + ksl.start: self.off + ksl.stop, msl]

        qc2a = pool.tile([128, KC * BL], F32, tag="qc2a")
        qc2 = pool.tile([128, KC * BL], F32, tag="qc2")
        if 'small' in phases:
            small_matmul(_W1Slice(1 * D), m1_8, qc2a)
            small_matmul(_W1Slice(2 * D), qb_sb, qc2, bias=bZ_sb,
                         accum_from=qc2a)
        else:
            nc.vector.memset(qc2[:, :], 0.0)

        feeder2 = None
        if 'zg' in phases:
            drain(z_g_phase(m1, wbm_sb, qc2, "b", [(0, 256)]))
            feeder2 = z_g_phase(m1, wbm_sb, qc2, "b2",
                                [(256, 256), (512, 256), (768, 256)])
        if 'scan' in phases:
            h2 = scan(t_steps, "b", feeder=feeder2, pump=2)
        else:
            h2 = scratch.tile([128, KC * BL], BF16, tag="h8", name="hstub_b")
            nc.vector.memset(h2[:, :], 0.0)
        if feeder2 is not None:
            drain(feeder2)
        m2, _ = mem_gru(h2, m1, m1_8, "b")

        nc.sync.dma_start(out=out[:, :], in_=m2[:, :])

        for p in (scratch, stream, pool):
            p.release()

    if split_waits:
        _split_multiwait(nc)
    return nc


_cache = {}


def _get_nc(t_steps=T):
    if t_steps not in _cache:
        _cache[t_steps] = _build(t_steps)
    return _cache[t_steps]


def _prep_inputs(c, q, Wb_w, W1_w, W1_b, W2_w, W2_b,
                 mem_Wih, mem_Whh, mem_bih, mem_bhh,
                 att_Wih, att_Whh, att_bih, att_bhh):
    """Host-side: transpose/cast/shard everything into per-core in_maps."""
    f32 = np.float32
    c = np.asarray(c, f32); q = np.asarray(q, f32)
    W1j = [np.asarray(W1_w[:, j * D:(j + 1) * D], f32) for j in range(9)]

    def _negz():  # [1, 3D] row: negate the z-gate columns of att weights
        s = np.ones((1, 3 * D), f32)
        s[:, D:2 * D] = -1.0
        return s

    def fold_bias(v):  # [D] -> [128, KC] (p, m)
        return np.ascontiguousarray(
            np.asarray(v, f32).reshape(KC, 128).T)

    def fold_bias3(bih, bhh, negate_z=False):  # [3D] -> [128, 3KC]
        v = np.asarray(bih, f32).copy()
        bhh = np.asarray(bhh, f32)
        v[:2 * D] += bhh[:2 * D]
        if negate_z:  # z-gate negated so the kernel gets 1-z from sigmoid
            v[D:2 * D] *= -1.0
        return np.ascontiguousarray(v.reshape(3 * KC, 128).T)

    shared = {
        "w1T": np.ascontiguousarray(np.asarray(W1_w, f32).T).astype(bf16_np),
        "w12T": np.ascontiguousarray((W1j[1] + W1j[2]).T).astype(bf16_np),
        "wbT": np.ascontiguousarray(np.asarray(Wb_w, f32).T).astype(bf16_np),
        "w2T": np.ascontiguousarray(np.asarray(W2_w, f32).T).astype(bf16_np),
        "aWihT": np.ascontiguousarray(
            np.asarray(att_Wih, f32).T * _negz()).astype(bf16_np),
        "aWhhT8": (np.ascontiguousarray(np.asarray(att_Whh, f32).T * _negz())
                   * S_WHH).astype(fp8_np),
        "mWihT": np.ascontiguousarray(np.asarray(mem_Wih, f32).T).astype(bf16_np),
        "mWhhT": np.ascontiguousarray(np.asarray(mem_Whh, f32).T).astype(bf16_np),
        "bZ": fold_bias(W1_b),
        "bG": fold_bias(W2_b),
        "bA": fold_bias3(att_bih, att_bhh, negate_z=True),
        "bM": fold_bias3(mem_bih, mem_bhh),
    }
    assert not np.any(np.asarray(att_bhh, f32)[2 * D:]), \
        "nonzero att_bhh n-gate bias not supported by this kernel build"
    assert not np.any(np.asarray(mem_bhh, f32)[2 * D:]), \
        "nonzero mem_bhh n-gate bias not supported by this kernel build"

    in_maps = []
    for ci in range(NCORES):
        s = ci * BL
        csh = c[:, s:s + BL, :].reshape(ROWS, D)
        qsh = q[s:s + BL, :]
        im = dict(shared)
        im["cT"] = np.ascontiguousarray(csh.T)
        im["qT"] = np.ascontiguousarray(
            qsh.reshape(BL, KC, 128).transpose(2, 1, 0).reshape(128, KC * BL))
        in_maps.append(im)
    return in_maps


def _unshard(results):
    m = np.empty((B, D), np.float32)
    for ci in range(NCORES):
        o = results[ci]["out"]  # [128, KC*BL]: [p, (k, b)]
        m[ci * BL:(ci + 1) * BL] = (
            o.reshape(128, KC, BL).transpose(2, 1, 0).reshape(BL, D))
    return m


def run_device(in_maps, trace=False):
    nc = _get_nc()
    res = run_bass_kernel_spmd(nc, in_maps, list(range(NCORES)), trace=trace)
    return res


def kernel(**inputs) -> np.ndarray:
    in_maps = _prep_inputs(**inputs)
    res = run_device(in_maps)
    return _unshard(res.results)


if __name__ == "__main__":
    np.random.seed(0)
    pass

